# revision 12
# baseline (speedup 1.0000x reference)
"""BinTokenizer kernel for Trainium2 (8 NeuronCores, data-parallel).

reference math: tokens = searchsorted(thresholds, clip(x, eps, 1-eps), 'right') - 1
with thresholds = linspace(0, 1, 257) in float32 == exactly i/256, so the
search reduces to floor(x * 256) (x*256 is exact in f32: power-of-two scale).

trn2's f32->int32 converts all round-to-nearest-even (DVE/ACT/DMA alike,
HW-probed), so floor needs care:

* fast path (the real inputs): jax.random.uniform f32 values all lie on the
  k*2^-23 grid, so v = x*256 is a multiple of 2^-15.  Then
  RNE(v - (0.5 - 2^-16)) == floor(v) exactly: the subtraction is exact (odd
  multiples of 2^-16 below 2^8 fit in f32's 24-bit significand) and the
  result sits 2^-16 away from the half-integer tie on the correct side.
  One DVE tensor_scalar per tile.  kernel() verifies the grid/range/
  threshold assumptions on the host and only then uses this path.

* general path (any f32): y0 = cvt(v); y = y0 - (cvt_f32(y0) > v), clamped
  to [0, 255].  Exact for every input regardless of convert rounding mode.

Each core handles 8 of the 64 batch rows; loads ride the SP HWDGE ring,
stores the ACT ring, one DVE op between them.  Measured ~420 GB/s aggregate
DMA (96% of the 435 GB/s SBUF fabric ceiling) per core.
"""

import sys

sys.path.insert(0, "/opt/trn_rl_repo")

import numpy as np

N_CORES = 8
B, T, D = 64, 4096, 512
PER_CORE = (B // N_CORES) * T * D  # 16,777,216 elements per core
P = 128                            # SBUF partitions
M = 8192                           # magic-path tile free dim (32 KiB/partition fp32)
M_GENERAL = 2048                   # general path holds 5 extra tmp tiles, so smaller

MAGIC = 0.5 - 2.0**-16

LAST_RESULT = None  # BassKernelResults of the most recent run (for test.py)

_program_cache = {}


def _build(variant: str, scale: float, t0: float, m: int):
    import concourse.bacc as bacc
    import concourse.tile as tile
    from concourse import mybir

    rows = PER_CORE // m
    ntiles = rows // P

    # Bacc (not raw Bass): Tile emits multi-wait instructions, and only
    # Bacc's generate_event_semaphores pass splits them to the TRN2
    # one-wait-per-instruction limit walrus enforces.
    nc = bacc.Bacc("TRN2")
    F32, I32, U8 = mybir.dt.float32, mybir.dt.int32, mybir.dt.uint8
    Alu = mybir.AluOpType
    x = nc.dram_tensor("x", [rows, m], F32, kind="ExternalInput")
    # tokens are 0..255: store uint8 on device (lossless; 4x less write
    # traffic), widen to int32 host-side while unsharding
    y = nc.dram_tensor("y", [rows, m], U8, kind="ExternalOutput")
    xt = x.rearrange("(n p) m -> n p m", p=P)
    yt = y.rearrange("(n p) m -> n p m", p=P)

    with tile.TileContext(nc) as tc:
        with tc.tile_pool(name="io_in", bufs=3) as in_pool, tc.tile_pool(
            name="io_out", bufs=3
        ) as out_pool, tc.tile_pool(name="tmp", bufs=2) as tmp_pool:
            # magic path: last tile processed in 4 small chunks so the tail
            # load->DVE->store serialization drains in ~2.5us instead of ~7us
            if variant == "magic":
                SUB = 4
                sub_m = m // SUB
                for i in range(ntiles):
                    if i < ntiles - 1:
                        t_in = in_pool.tile([P, m], F32, tag="in")
                        nc.sync.dma_start(t_in[:], xt[i])
                        t_out = out_pool.tile([P, m], U8, tag="out")
                        # token = RNE(x*scale - MAGIC) == floor(x*scale) on grid
                        nc.vector.tensor_scalar(
                            t_out[:], t_in[:], float(scale), MAGIC,
                            Alu.mult, Alu.subtract,
                        )
                        nc.scalar.dma_start(yt[i], t_out[:])
                    else:
                        for j in range(SUB):
                            cols = slice(j * sub_m, (j + 1) * sub_m)
                            t_in = in_pool.tile([P, sub_m], F32, tag="in_s")
                            nc.sync.dma_start(t_in[:], xt[i][:, cols])
                            t_out = out_pool.tile([P, sub_m], U8, tag="out_s")
                            nc.vector.tensor_scalar(
                                t_out[:], t_in[:], float(scale), MAGIC,
                                Alu.mult, Alu.subtract,
                            )
                            nc.scalar.dma_start(yt[i][:, cols], t_out[:])
            for i in range(ntiles if variant != "magic" else 0):
                t_in = in_pool.tile([P, m], F32, tag="in")
                nc.sync.dma_start(t_in[:], xt[i])
                t_out = out_pool.tile([P, m], U8, tag="out")
                if True:
                    # v = (x - t0) * scale ; y0 = cvt(v)
                    t_v = tmp_pool.tile([P, m], F32, tag="v")
                    if t0 == 0.0:
                        nc.vector.tensor_scalar(
                            t_v[:], t_in[:], float(scale), None, Alu.mult
                        )
                    else:
                        nc.vector.tensor_scalar(
                            t_v[:], t_in[:], float(t0), float(scale),
                            Alu.subtract, Alu.mult,
                        )
                    t_y0 = tmp_pool.tile([P, m], I32, tag="y0")
                    nc.vector.tensor_scalar(t_y0[:], t_v[:], 1.0, None, Alu.mult)
                    # y0 back to f32 on the (otherwise idle) ACT engine
                    t_y0f = tmp_pool.tile([P, m], F32, tag="y0f")
                    nc.scalar.activation(
                        t_y0f[:], t_y0[:], mybir.ActivationFunctionType.Copy
                    )
                    t_gt = tmp_pool.tile([P, m], I32, tag="gt")
                    nc.vector.tensor_tensor(t_gt[:], t_y0f[:], t_v[:], Alu.is_gt)
                    t_y1 = tmp_pool.tile([P, m], I32, tag="y1")
                    nc.vector.tensor_tensor(t_y1[:], t_y0[:], t_gt[:], Alu.subtract)
                    nc.vector.tensor_scalar(
                        t_out[:], t_y1[:], 255, 0, Alu.min, Alu.max
                    )  # clamp keeps the u8 convert in-range for any input
                # stores on the ACT HWDGE ring so they don't queue behind loads
                nc.scalar.dma_start(yt[i], t_out[:])

    nc.finalize()  # Bacc pass pipeline (reg alloc, event-sem wait splitting)
    return nc


def _fast_path_ok(x: np.ndarray, t: np.ndarray) -> bool:
    """Assumptions behind the single-op magic kernel: thresholds are exactly
    the i/256 grid, and every input is a k*2^-23 threefry-uniform in [0,1)."""
    if t.shape != (257,) or not np.array_equal(
        t.astype(np.float64), np.arange(257) / 256.0
    ):
        return False
    flat = x.reshape(-1)
    step = 1 << 24
    for i in range(0, flat.size, step):
        c = flat[i : i + step].astype(np.float64)
        if c.size == 0:
            continue
        lo, hi = c.min(), c.max()
        if lo < 0.0 or hi >= 1.0:
            return False
        if np.any(np.mod(c * 8388608.0, 1.0) != 0.0):
            return False
    return True


def kernel(inputs: np.ndarray, thresholds: np.ndarray) -> np.ndarray:
    global LAST_RESULT
    from concourse.bass_utils import run_bass_kernel_spmd

    x = np.asarray(inputs, dtype=np.float32)
    if not x.flags.c_contiguous:
        x = np.ascontiguousarray(x)
    t = np.asarray(thresholds, dtype=np.float32)

    td = t.astype(np.float64)
    scale = float(1.0 / (td[1] - td[0]))
    t0 = float(td[0])

    variant = "magic" if (scale == 256.0 and t0 == 0.0 and _fast_path_ok(x, t)) else "general"
    m = M if variant == "magic" else M_GENERAL
    rows = PER_CORE // m

    key = (variant, scale, t0)
    if key not in _program_cache:
        _program_cache[key] = _build(variant, scale, t0, m)
    nc = _program_cache[key]

    shards = x.reshape(N_CORES, rows, m)
    in_maps = [{"x": shards[c]} for c in range(N_CORES)]

    res = run_bass_kernel_spmd(nc, in_maps, list(range(N_CORES)))
    LAST_RESULT = res

    out = np.empty((N_CORES, rows, m), dtype=np.int32)
    for c in range(N_CORES):
        out[c] = res.results[c]["y"]
    return out.reshape(B, T, D)


# revision 13
# speedup vs baseline: 1.0042x; 1.0042x over previous
"""BinTokenizer kernel for Trainium2 (8 NeuronCores, data-parallel).

reference math: tokens = searchsorted(thresholds, clip(x, eps, 1-eps), 'right') - 1
with thresholds = linspace(0, 1, 257) in float32 == exactly i/256, so the
search reduces to floor(x * 256) (x*256 is exact in f32: power-of-two scale).

trn2's f32->int32 converts all round-to-nearest-even (DVE/ACT/DMA alike,
HW-probed), so floor needs care:

* fast path (the real inputs): jax.random.uniform f32 values all lie on the
  k*2^-23 grid, so v = x*256 is a multiple of 2^-15.  Then
  RNE(v - (0.5 - 2^-16)) == floor(v) exactly: the subtraction is exact (odd
  multiples of 2^-16 below 2^8 fit in f32's 24-bit significand) and the
  result sits 2^-16 away from the half-integer tie on the correct side.
  One DVE tensor_scalar per tile.  kernel() verifies the grid/range/
  threshold assumptions on the host and only then uses this path.

* general path (any f32): y0 = cvt(v); y = y0 - (cvt_f32(y0) > v), clamped
  to [0, 255].  Exact for every input regardless of convert rounding mode.

Each core handles 8 of the 64 batch rows; loads ride the SP HWDGE ring,
stores the ACT ring, one DVE op between them.  Measured ~420 GB/s aggregate
DMA (96% of the 435 GB/s SBUF fabric ceiling) per core.
"""

import sys

sys.path.insert(0, "/opt/trn_rl_repo")

import numpy as np

N_CORES = 8
B, T, D = 64, 4096, 512
PER_CORE = (B // N_CORES) * T * D  # 16,777,216 elements per core
P = 128                            # SBUF partitions
M = 8192                           # magic-path tile free dim (32 KiB/partition fp32)
M_GENERAL = 2048                   # general path holds 5 extra tmp tiles, so smaller

MAGIC = 0.5 - 2.0**-16

LAST_RESULT = None  # BassKernelResults of the most recent run (for test.py)

_program_cache = {}


def _build(variant: str, scale: float, t0: float, m: int):
    import concourse.bacc as bacc
    import concourse.tile as tile
    from concourse import mybir

    rows = PER_CORE // m
    ntiles = rows // P

    # Bacc (not raw Bass): Tile emits multi-wait instructions, and only
    # Bacc's generate_event_semaphores pass splits them to the TRN2
    # one-wait-per-instruction limit walrus enforces.
    nc = bacc.Bacc("TRN2")
    F32, I32, U8 = mybir.dt.float32, mybir.dt.int32, mybir.dt.uint8
    Alu = mybir.AluOpType
    x = nc.dram_tensor("x", [rows, m], F32, kind="ExternalInput")
    # tokens are 0..255: store uint8 on device (lossless; 4x less write
    # traffic), widen to int32 host-side while unsharding
    y = nc.dram_tensor("y", [rows, m], U8, kind="ExternalOutput")
    xt = x.rearrange("(n p) m -> n p m", p=P)
    yt = y.rearrange("(n p) m -> n p m", p=P)

    with tile.TileContext(nc) as tc:
        with tc.tile_pool(name="io_in", bufs=4) as in_pool, tc.tile_pool(
            name="io_out", bufs=3
        ) as out_pool, tc.tile_pool(name="tmp", bufs=2) as tmp_pool:
            # magic path: last tile processed in 4 small chunks so the tail
            # load->DVE->store serialization drains in ~2.5us instead of ~7us
            if variant == "magic":
                SUB = 4
                sub_m = m // SUB
                for i in range(ntiles):
                    if i < ntiles - 1:
                        t_in = in_pool.tile([P, m], F32, tag="in")
                        nc.sync.dma_start(t_in[:], xt[i])
                        t_out = out_pool.tile([P, m], U8, tag="out")
                        # token = RNE(x*scale - MAGIC) == floor(x*scale) on grid
                        nc.vector.tensor_scalar(
                            t_out[:], t_in[:], float(scale), MAGIC,
                            Alu.mult, Alu.subtract,
                        )
                        nc.scalar.dma_start(yt[i], t_out[:])
                    else:
                        for j in range(SUB):
                            cols = slice(j * sub_m, (j + 1) * sub_m)
                            t_in = in_pool.tile([P, sub_m], F32, tag="in_s")
                            nc.sync.dma_start(t_in[:], xt[i][:, cols])
                            t_out = out_pool.tile([P, sub_m], U8, tag="out_s")
                            nc.vector.tensor_scalar(
                                t_out[:], t_in[:], float(scale), MAGIC,
                                Alu.mult, Alu.subtract,
                            )
                            nc.scalar.dma_start(yt[i][:, cols], t_out[:])
            for i in range(ntiles if variant != "magic" else 0):
                t_in = in_pool.tile([P, m], F32, tag="in")
                nc.sync.dma_start(t_in[:], xt[i])
                t_out = out_pool.tile([P, m], U8, tag="out")
                if True:
                    # v = (x - t0) * scale ; y0 = cvt(v)
                    t_v = tmp_pool.tile([P, m], F32, tag="v")
                    if t0 == 0.0:
                        nc.vector.tensor_scalar(
                            t_v[:], t_in[:], float(scale), None, Alu.mult
                        )
                    else:
                        nc.vector.tensor_scalar(
                            t_v[:], t_in[:], float(t0), float(scale),
                            Alu.subtract, Alu.mult,
                        )
                    t_y0 = tmp_pool.tile([P, m], I32, tag="y0")
                    nc.vector.tensor_scalar(t_y0[:], t_v[:], 1.0, None, Alu.mult)
                    # y0 back to f32 on the (otherwise idle) ACT engine
                    t_y0f = tmp_pool.tile([P, m], F32, tag="y0f")
                    nc.scalar.activation(
                        t_y0f[:], t_y0[:], mybir.ActivationFunctionType.Copy
                    )
                    t_gt = tmp_pool.tile([P, m], I32, tag="gt")
                    nc.vector.tensor_tensor(t_gt[:], t_y0f[:], t_v[:], Alu.is_gt)
                    t_y1 = tmp_pool.tile([P, m], I32, tag="y1")
                    nc.vector.tensor_tensor(t_y1[:], t_y0[:], t_gt[:], Alu.subtract)
                    nc.vector.tensor_scalar(
                        t_out[:], t_y1[:], 255, 0, Alu.min, Alu.max
                    )  # clamp keeps the u8 convert in-range for any input
                # stores on the ACT HWDGE ring so they don't queue behind loads
                nc.scalar.dma_start(yt[i], t_out[:])

    nc.finalize()  # Bacc pass pipeline (reg alloc, event-sem wait splitting)
    return nc


def _fast_path_ok(x: np.ndarray, t: np.ndarray) -> bool:
    """Assumptions behind the single-op magic kernel: thresholds are exactly
    the i/256 grid, and every input is a k*2^-23 threefry-uniform in [0,1)."""
    if t.shape != (257,) or not np.array_equal(
        t.astype(np.float64), np.arange(257) / 256.0
    ):
        return False
    flat = x.reshape(-1)
    step = 1 << 24
    for i in range(0, flat.size, step):
        c = flat[i : i + step].astype(np.float64)
        if c.size == 0:
            continue
        lo, hi = c.min(), c.max()
        if lo < 0.0 or hi >= 1.0:
            return False
        if np.any(np.mod(c * 8388608.0, 1.0) != 0.0):
            return False
    return True


def kernel(inputs: np.ndarray, thresholds: np.ndarray) -> np.ndarray:
    global LAST_RESULT
    from concourse.bass_utils import run_bass_kernel_spmd

    x = np.asarray(inputs, dtype=np.float32)
    if not x.flags.c_contiguous:
        x = np.ascontiguousarray(x)
    t = np.asarray(thresholds, dtype=np.float32)

    td = t.astype(np.float64)
    scale = float(1.0 / (td[1] - td[0]))
    t0 = float(td[0])

    variant = "magic" if (scale == 256.0 and t0 == 0.0 and _fast_path_ok(x, t)) else "general"
    m = M if variant == "magic" else M_GENERAL
    rows = PER_CORE // m

    key = (variant, scale, t0)
    if key not in _program_cache:
        _program_cache[key] = _build(variant, scale, t0, m)
    nc = _program_cache[key]

    shards = x.reshape(N_CORES, rows, m)
    in_maps = [{"x": shards[c]} for c in range(N_CORES)]

    res = run_bass_kernel_spmd(nc, in_maps, list(range(N_CORES)))
    LAST_RESULT = res

    out = np.empty((N_CORES, rows, m), dtype=np.int32)
    for c in range(N_CORES):
        out[c] = res.results[c]["y"]
    return out.reshape(B, T, D)
